# revision 18
# baseline (speedup 1.0000x reference)
"""Multi-head self-attention on 8 Trainium2 NeuronCores.

Problem: x:(4,2048,1024) fp32; q = x@Wq, kv = x@Wkv (k,v split), 8 heads of
dim 64, softmax(q k^T / 8) v, concat heads, @Wo + bo -> (4,2048,1024).

Sharding: core c handles batch b=c//2 and head group g=c%2 (4 of 8 heads).
Each core computes its batch's projections restricted to its 4 heads, full
attention for those heads, and a partial output projection y_c = U_norm @ Wo_g.
Host gathers: out[b] = y_{2b} + y_{2b+1} + bo  (the "all-reduce" of the
tensor-parallel head split, done at unshard time).

Device algorithm (per core), all matmul operands fp16, PSUM accumulate fp32:
  - host supplies xT = x[b].T so the contraction dim (QDIM) is the partition
    axis; projections compute qT/kT (head_dim-major) and v (seq-major) tiles.
  - attention per head, per i-half (1024 q rows), per j-tile (128 k rows):
      simT[j,i] = kT_h(j)^T-tile @ qT_h        (PE, K=64)
      expT = exp(SCALE*simT)                   (ACT, reads PSUM directly)
      U~[d,i] += [v_h | 1]^T @ expT            (PE, K=128; row 64 = softmax sum)
    then normalization: r = 1/s via fast-reciprocal (DVE), R = ones x r
    broadcast (PE K=1 matmul), U_norm = U~ * R (DVE).
  - y[m,:] = U_norm_pairs^T @ Wo_g (K=128 per head pair), DVE drain, DMA out.
"""

import numpy as np

# ---- problem constants (hardcoded per the harness contract) ----
B, N, QDIM = 4, 2048, 1024
HEADS, DIM_MODEL = 8, 512
HEAD_DIM = DIM_MODEL // HEADS  # 64
SCALE = HEAD_DIM ** -0.5  # 0.125
N_CORES = 8
HEADS_PER_CORE = HEADS // 2  # 4 (head-group split across 2 cores per batch)
DMC = HEADS_PER_CORE * HEAD_DIM  # 256 per-core model dim slice


def build_nc(seq=N, qd=QDIM, nh=HEADS_PER_CORE, hd=HEAD_DIM, dout=QDIM,
             scale=SCALE, ihw=1024, skip_norm=False, norm_mode='dve',
             expp_bufs=3, upool_bufs=2, rows_bufs=2, ysb_bufs=3,
             phases='all', simp_bufs=2, uaccp_bufs=1, spare_bufs=2,
             xt_one_dma=True, y_pair_dma=True):
    """Build the per-core Bass program (same program on all 8 cores)."""
    from contextlib import ExitStack

    import concourse.bass as bass
    import concourse.tile as tile
    from concourse import bacc, mybir

    P = 128
    NC5 = 512  # psum bank width in fp32
    f16 = mybir.dt.float16
    f32 = mybir.dt.float32
    Exp = mybir.ActivationFunctionType.Exp
    Ln = mybir.ActivationFunctionType.Ln

    dmc = nh * hd                 # per-core projected dim (256)
    kt = qd // P                  # contraction tiles over QDIM (8)
    seqt = seq // P               # seq tiles (16)
    mtiles = max(1, dmc // P)     # qT/kT partition tiles (2)
    heads_per_mtile = nh // mtiles
    ihw = min(ihw, seq)           # i-half width
    n_ih = seq // ihw
    npairs = mtiles               # head pairs stacked for final proj (2)

    def chunks(total, w=NC5):
        c0 = 0
        while c0 < total:
            yield c0, min(w, total - c0)
            c0 += w

    nc = bacc.Bacc("TRN2", target_bir_lowering=False, debug=False,
                   num_devices=N_CORES)

    xt = nc.dram_tensor("xt", (qd, seq), f16, kind="ExternalInput").ap()
    wq = nc.dram_tensor("wq", (qd, dmc), f16, kind="ExternalInput").ap()
    wk = nc.dram_tensor("wk", (qd, dmc), f16, kind="ExternalInput").ap()
    wv = nc.dram_tensor("wv", (qd, dmc), f16, kind="ExternalInput").ap()
    wo = nc.dram_tensor("wo", (dmc, dout), f16, kind="ExternalInput").ap()
    y = nc.dram_tensor("y", (seq, dout), f32, kind="ExternalOutput").ap()

    with tile.TileContext(nc) as tc, ExitStack() as ctx:
        # ---- SBUF pools ----
        persist = ctx.enter_context(tc.tile_pool(name="persist", bufs=1))
        expp = ctx.enter_context(tc.tile_pool(name="expp", bufs=expp_bufs))
        upool = ctx.enter_context(tc.tile_pool(name="upool", bufs=upool_bufs))
        rows = ctx.enter_context(tc.tile_pool(name="rows", bufs=rows_bufs))
        ysb = ctx.enter_context(tc.tile_pool(name="ysb", bufs=ysb_bufs))
        # ---- PSUM pools (8 banks total: 2 spare + 4 sim + 2 uacc) ----
        spare = ctx.enter_context(tc.tile_pool(name="spare", bufs=spare_bufs, space="PSUM"))
        simp = ctx.enter_context(tc.tile_pool(name="simp", bufs=simp_bufs, space="PSUM"))
        uaccp = ctx.enter_context(tc.tile_pool(name="uaccp", bufs=uaccp_bufs, space="PSUM"))

        # ---- persistent SBUF tensors ----
        xt_sb = persist.tile([P, kt, seq], f16)
        wq_sb = persist.tile([P, kt, dmc], f16)
        wk_sb = persist.tile([P, kt, dmc], f16)
        wv_sb = persist.tile([P, kt, dmc], f16)
        wo_sb = persist.tile([min(P, dmc), npairs, dout], f16)
        v_sb = persist.tile([P, seqt, nh, hd + 1], f16)
        qt_sb = persist.tile([min(P, dmc), mtiles, seq], f16)
        kt_sb = persist.tile([min(P, dmc), mtiles, seq], f16)
        upairs = [persist.tile([min(P, dmc), seq], f16, name=f"upair{p}")
                  for p in range(npairs)]
        ones65 = persist.tile([65, hd], f16)

        # ---- input loads ----
        if xt_one_dma:
            nc.sync.dma_start(xt_sb[:], xt.rearrange("(ko ki) s -> ki ko s",
                                                     ki=P))
        else:
            for ko in range(kt):
                nc.sync.dma_start(xt_sb[:, ko, :], xt[ko * P:(ko + 1) * P, :])
        nc.sync.dma_start(wq_sb[:], wq.rearrange("(ko ki) m -> ki ko m", ki=P))
        nc.sync.dma_start(wk_sb[:], wk.rearrange("(ko ki) m -> ki ko m", ki=P))
        nc.sync.dma_start(wv_sb[:], wv.rearrange("(ko ki) m -> ki ko m", ki=P))
        nc.sync.dma_start(wo_sb[:], wo.rearrange("(t p) n -> p t n", p=min(P, dmc)))
        nc.vector.memset(v_sb[:, :, :, hd:hd + 1], 1.0)
        nc.vector.memset(ones65[:], 1.0)

        def proj_kq_tile(mt, which, n0, nw):
            """One [mp, nw] tile of kT (which=0) or qT (which=1) for m-tile mt."""
            mp = min(P, dmc)
            w_sb, out_sb = ((wk_sb, kt_sb), (wq_sb, qt_sb))[which]
            ps = spare.tile([mp, NC5], f32, tag="ps512", name="ps")
            for ko in range(kt):
                nc.tensor.matmul(
                    ps[:, 0:nw],
                    lhsT=w_sb[:, ko, mt * mp:(mt + 1) * mp],
                    rhs=xt_sb[:, ko, n0:n0 + nw],
                    start=(ko == 0), stop=(ko == kt - 1))
            nc.vector.tensor_copy(
                out_sb[0:mp, mt, n0:n0 + nw], ps[:, 0:nw])

        def proj_v_tile(jt):
            """v natural layout [seq, dmc] -> v_sb[:, jt, h, 0:hd]."""
            ps = spare.tile([P, dmc], f32, tag="ps512", name="ps")
            for ko in range(kt):
                nc.tensor.matmul(
                    ps[:],
                    lhsT=xt_sb[:, ko, jt * P:(jt + 1) * P],
                    rhs=wv_sb[:, ko, :],
                    start=(ko == 0), stop=(ko == kt - 1))
            nc.vector.tensor_copy(
                v_sb[:, jt, :, 0:hd],
                ps.rearrange("p (h d) -> p h d", h=nh))

        def attn_head_ih(h, ih):
            if True:
                mt = h // heads_per_mtile
                hb = (h % heads_per_mtile) * hd
                pair = h // heads_per_mtile
                i0 = ih * ihw
                uacc = uaccp.tile([hd + 1, ihw], f32, tag="uacc")
                for jt in range(seqt):
                    sim = simp.tile([P, ihw], f32, tag="sim")
                    for c0, cw in chunks(ihw):
                        nc.tensor.matmul(
                            sim[:, c0:c0 + cw],
                            lhsT=kt_sb[hb:hb + hd, mt, jt * P:(jt + 1) * P],
                            rhs=qt_sb[hb:hb + hd, mt, i0 + c0:i0 + c0 + cw],
                            start=True, stop=True)
                    expt = expp.tile([P, ihw], f16, tag="expt")
                    nc.scalar.activation(expt[:], sim[:], Exp, scale=scale)
                    for c0, cw in chunks(ihw):
                        nc.tensor.matmul(
                            uacc[:, c0:c0 + cw],
                            lhsT=v_sb[:, jt, h, :],
                            rhs=expt[:, c0:c0 + cw],
                            start=(jt == 0), stop=(jt == seqt - 1))
                # normalization: r = 1/s; U_norm = U~ * broadcast(r)
                u_sb = upool.tile([hd, ihw], f16, tag="u")
                nc.vector.tensor_copy(u_sb[:], uacc[0:hd, :])
                if skip_norm:
                    nc.vector.tensor_copy(
                        upairs[pair][hb:hb + hd, i0:i0 + ihw], u_sb[:])
                    return
                srow = rows.tile([65, ihw], f32, tag="srow")
                nc.vector.tensor_copy(srow[64:65, :], uacc[hd:hd + 1, :])
                # r = 1/s. The fused custom-DVE reciprocal op returns garbage
                # on this HW path, so either ACT ln/exp ("ln") or a manual
                # Newton iteration from standard DVE ops ("dve", default —
                # keeps the critical ACT engine free for the softmax exps).
                rrow = rows.tile([65, ihw], f32, tag="rrow")
                rrow16 = rows.tile([65, ihw], f16, tag="rrow16")
                if norm_mode == "ln":
                    lnrow = rows.tile([65, ihw], f32, tag="lnrow")
                    nc.scalar.activation(lnrow[64:65, :], srow[64:65, :], Ln)
                    nc.scalar.activation(rrow[64:65, :], lnrow[64:65, :], Exp,
                                         scale=-1.0)
                    nc.vector.tensor_copy(rrow16[64:65, :], rrow[64:65, :])
                elif norm_mode == "dve":
                    i32 = mybir.dt.int32
                    s_r, u_r, t_r = (srow[64:65, :], rrow[64:65, :],
                                     None)
                    trow = rows.tile([65, ihw], f32, tag="trow")
                    t_r = trow[64:65, :]
                    # u0 = bitcast(~bits(s)) * 0.23549792   (u = -1/s approx)
                    nc.vector.tensor_scalar(t_r.bitcast(i32), s_r.bitcast(i32),
                                            -1, None,
                                            op0=mybir.AluOpType.bitwise_xor)
                    nc.vector.tensor_scalar_mul(u_r, t_r, 0.23549792)
                    # two Newton passes: u <- (s*u + c)*u, c = 2.0017324, 2.0
                    for c in (2.0017324, 2.0):
                        nc.vector.tensor_mul(t_r, s_r, u_r)
                        nc.vector.scalar_tensor_tensor(
                            u_r, t_r, float(c), u_r,
                            op0=mybir.AluOpType.add, op1=mybir.AluOpType.mult)
                    # r = -u, cast to fp16
                    nc.vector.tensor_scalar_mul(rrow16[64:65, :], u_r, -1.0)
                elif norm_mode == "copy":  # timing-only bisect: wrong math
                    nc.vector.tensor_copy(rrow16[64:65, :], srow[64:65, :])
                else:
                    raise ValueError(norm_mode)
                for c0, cw in chunks(ihw):
                    rps = spare.tile([hd, NC5], f32, tag="ps512")
                    nc.tensor.matmul(
                        rps[:, 0:cw],
                        lhsT=ones65[64:65, :],
                        rhs=rrow16[64:65, c0:c0 + cw],
                        start=True, stop=True)
                    nc.vector.tensor_mul(
                        upairs[pair][hb:hb + hd, i0 + c0:i0 + c0 + cw],
                        u_sb[:, c0:c0 + cw], rps[:, 0:cw])

        def final_proj():
            mp = min(P, dmc)
            for m in range(seqt):
                if y_pair_dma:
                    yt = ysb.tile([P, dout], f32, tag="yt")
                for n0, nw in chunks(dout):
                    yps = spare.tile([P, NC5], f32, tag="ps512")
                    for p in range(npairs):
                        nc.tensor.matmul(
                            yps[:, 0:nw],
                            lhsT=upairs[p][0:mp, m * P:(m + 1) * P],
                            rhs=wo_sb[0:mp, p, n0:n0 + nw],
                            start=(p == 0), stop=(p == npairs - 1))
                    if y_pair_dma:
                        nc.vector.tensor_copy(yt[:, n0:n0 + nw], yps[:, 0:nw])
                    else:
                        yt = ysb.tile([P, NC5], f32, tag="yt")
                        nc.vector.tensor_copy(yt[:, 0:nw], yps[:, 0:nw])
                        nc.sync.dma_start(
                            y[m * P:(m + 1) * P, n0:n0 + nw], yt[:, 0:nw])
                if y_pair_dma:
                    nc.sync.dma_start(y[m * P:(m + 1) * P, :], yt[:])

        # Emission schedule: per-engine instruction order is static after
        # scheduling, so projection granules are threaded between attention
        # (h, ih) phases — each phase's inputs emitted one phase ahead; the
        # ACT-paced attention then hides the remaining projection PE work.
        attn_phases = [(h, ih) for h in range(nh) for ih in range(n_ih)]

        def phase_needs(idx):
            # granules that must be emitted before attention phase idx;
            # every phase's j-loop consumes ALL v tiles, so v has deadline 0.
            if idx >= len(attn_phases):
                return []
            h, ih = attn_phases[idx]
            mt = h // heads_per_mtile
            need = [("k", mt, n0, nw) for n0, nw in chunks(seq)]
            need += [("q", mt, n0, nw) for n0, nw in chunks(seq)
                     if n0 < (ih + 1) * ihw and n0 + nw > ih * ihw]
            if idx == 0:
                need += [("v", jt) for jt in range(seqt)]
            return need

        emitted = set()

        def emit_granules(needs):
            for g in needs:
                if g in emitted:
                    continue
                emitted.add(g)
                if g[0] == "v":
                    proj_v_tile(g[1])
                else:
                    which = 0 if g[0] == "k" else 1
                    proj_kq_tile(g[1], which, g[2], g[3])

        all_granules = []
        for idx in range(len(attn_phases)):
            for g in phase_needs(idx):
                if g not in all_granules:
                    all_granules.append(g)

        if phases == 'proj':
            emit_granules(all_granules)
        else:
            emit_granules(phase_needs(0))
            # deadline-ordered backlog, spread evenly across early boundaries
            backlog = [g for g in all_granules if g not in emitted]
            nb = max(1, len(attn_phases) - 2)
            share = -(-len(backlog) // nb)
            for idx, (h, ih) in enumerate(attn_phases):
                attn_head_ih(h, ih)
                emit_granules(phase_needs(idx + 1))
                take = [g for g in backlog if g not in emitted][:share]
                emit_granules(take)
            if phases == 'all':
                final_proj()

    nc.compile()
    return nc


_NC_CACHE = {}


def _get_nc():
    if "nc" not in _NC_CACHE:
        _NC_CACHE["nc"] = build_nc()
    return _NC_CACHE["nc"]


def _prep_core_inputs(x, Wq, Wkv, Wo):
    """Host-side shard + layout prep: per-core fp16 slices."""
    f16 = np.float16
    in_maps = []
    for c in range(N_CORES):
        b, g = c // 2, c % 2
        s = slice(g * DMC, (g + 1) * DMC)
        in_maps.append({
            "xt": np.ascontiguousarray(x[b].T).astype(f16),
            "wq": np.ascontiguousarray(Wq[:, s]).astype(f16),
            "wk": np.ascontiguousarray(Wkv[:, g * DMC:(g + 1) * DMC]).astype(f16),
            "wv": np.ascontiguousarray(
                Wkv[:, DIM_MODEL + g * DMC:DIM_MODEL + (g + 1) * DMC]).astype(f16),
            "wo": np.ascontiguousarray(Wo[s, :]).astype(f16),
        })
    return in_maps


def kernel(x, Wq, Wkv, Wo, bo):
    from concourse import bass_utils

    x = np.asarray(x, dtype=np.float32)
    Wq = np.asarray(Wq, dtype=np.float32)
    Wkv = np.asarray(Wkv, dtype=np.float32)
    Wo = np.asarray(Wo, dtype=np.float32)
    bo = np.asarray(bo, dtype=np.float32)

    nc = _get_nc()
    in_maps = _prep_core_inputs(x, Wq, Wkv, Wo)
    res = bass_utils.run_bass_kernel_spmd(nc, in_maps,
                                          core_ids=list(range(N_CORES)))
    out = np.empty((B, N, QDIM), dtype=np.float32)
    for b in range(B):
        out[b] = res.results[2 * b]["y"] + res.results[2 * b + 1]["y"] + bo
    return out


# revision 19
# speedup vs baseline: 1.0379x; 1.0379x over previous
"""Multi-head self-attention on 8 Trainium2 NeuronCores.

Problem: x:(4,2048,1024) fp32; q = x@Wq, kv = x@Wkv (k,v split), 8 heads of
dim 64, softmax(q k^T / 8) v, concat heads, @Wo + bo -> (4,2048,1024).

Sharding: core c handles batch b=c//2 and head group g=c%2 (4 of 8 heads).
Each core computes its batch's projections restricted to its 4 heads, full
attention for those heads, and a partial output projection y_c = U_norm @ Wo_g.
Host gathers: out[b] = y_{2b} + y_{2b+1} + bo  (the "all-reduce" of the
tensor-parallel head split, done at unshard time).

Device algorithm (per core), all matmul operands fp16, PSUM accumulate fp32:
  - host supplies xT = x[b].T so the contraction dim (QDIM) is the partition
    axis; projections compute qT/kT (head_dim-major) and v (seq-major) tiles.
  - attention per head, per i-half (1024 q rows), per j-tile (128 k rows):
      simT[j,i] = kT_h(j)^T-tile @ qT_h        (PE, K=64)
      expT = exp(SCALE*simT)                   (ACT, reads PSUM directly)
      U~[d,i] += [v_h | 1]^T @ expT            (PE, K=128; row 64 = softmax sum)
    then normalization: r = 1/s via fast-reciprocal (DVE), R = ones x r
    broadcast (PE K=1 matmul), U_norm = U~ * R (DVE).
  - y[m,:] = U_norm_pairs^T @ Wo_g (K=128 per head pair), DVE drain, DMA out.
"""

import numpy as np

# ---- problem constants (hardcoded per the harness contract) ----
B, N, QDIM = 4, 2048, 1024
HEADS, DIM_MODEL = 8, 512
HEAD_DIM = DIM_MODEL // HEADS  # 64
SCALE = HEAD_DIM ** -0.5  # 0.125
N_CORES = 8
HEADS_PER_CORE = HEADS // 2  # 4 (head-group split across 2 cores per batch)
DMC = HEADS_PER_CORE * HEAD_DIM  # 256 per-core model dim slice


def build_nc(seq=N, qd=QDIM, nh=HEADS_PER_CORE, hd=HEAD_DIM, dout=QDIM,
             scale=SCALE, ihw=1024, skip_norm=False, norm_mode='dve',
             expp_bufs=3, upool_bufs=2, rows_bufs=2, ysb_bufs=3,
             phases='all', simp_bufs=2, uaccp_bufs=1, spare_bufs=2,
             xt_one_dma=True, y_pair_dma=True):
    """Build the per-core Bass program (same program on all 8 cores)."""
    from contextlib import ExitStack

    import concourse.bass as bass
    import concourse.tile as tile
    from concourse import bacc, mybir

    P = 128
    NC5 = 512  # psum bank width in fp32
    f16 = mybir.dt.float16
    f32 = mybir.dt.float32
    Exp = mybir.ActivationFunctionType.Exp
    Ln = mybir.ActivationFunctionType.Ln

    dmc = nh * hd                 # per-core projected dim (256)
    kt = qd // P                  # contraction tiles over QDIM (8)
    seqt = seq // P               # seq tiles (16)
    mtiles = max(1, dmc // P)     # qT/kT partition tiles (2)
    heads_per_mtile = nh // mtiles
    ihw = min(ihw, seq)           # i-half width
    n_ih = seq // ihw
    npairs = mtiles               # head pairs stacked for final proj (2)

    def chunks(total, w=NC5):
        c0 = 0
        while c0 < total:
            yield c0, min(w, total - c0)
            c0 += w

    nc = bacc.Bacc("TRN2", target_bir_lowering=False, debug=False,
                   num_devices=N_CORES)

    xt = nc.dram_tensor("xt", (qd, seq), f16, kind="ExternalInput").ap()
    wq = nc.dram_tensor("wq", (qd, dmc), f16, kind="ExternalInput").ap()
    wk = nc.dram_tensor("wk", (qd, dmc), f16, kind="ExternalInput").ap()
    wv = nc.dram_tensor("wv", (qd, dmc), f16, kind="ExternalInput").ap()
    wo = nc.dram_tensor("wo", (dmc, dout), f16, kind="ExternalInput").ap()
    y = nc.dram_tensor("y", (seq, dout), f32, kind="ExternalOutput").ap()

    with tile.TileContext(nc) as tc, ExitStack() as ctx:
        # ---- SBUF pools ----
        persist = ctx.enter_context(tc.tile_pool(name="persist", bufs=1))
        expp = ctx.enter_context(tc.tile_pool(name="expp", bufs=expp_bufs))
        upool = ctx.enter_context(tc.tile_pool(name="upool", bufs=upool_bufs))
        rows = ctx.enter_context(tc.tile_pool(name="rows", bufs=rows_bufs))
        ysb = ctx.enter_context(tc.tile_pool(name="ysb", bufs=ysb_bufs))
        # ---- PSUM pools (8 banks total: 2 spare + 4 sim + 2 uacc) ----
        spare = ctx.enter_context(tc.tile_pool(name="spare", bufs=spare_bufs, space="PSUM"))
        simp = ctx.enter_context(tc.tile_pool(name="simp", bufs=simp_bufs, space="PSUM"))
        uaccp = ctx.enter_context(tc.tile_pool(name="uaccp", bufs=uaccp_bufs, space="PSUM"))

        # ---- persistent SBUF tensors ----
        xt_sb = persist.tile([P, kt, seq], f16)
        wq_sb = persist.tile([P, kt, dmc], f16)
        wk_sb = persist.tile([P, kt, dmc], f16)
        wv_sb = persist.tile([P, kt, dmc], f16)
        wo_sb = persist.tile([min(P, dmc), npairs, dout], f16)
        v_sb = persist.tile([P, seqt, nh, hd + 1], f16)
        qt_sb = persist.tile([min(P, dmc), mtiles, seq], f16)
        kt_sb = persist.tile([min(P, dmc), mtiles, seq], f16)
        upairs = [persist.tile([min(P, dmc), seq], f16, name=f"upair{p}")
                  for p in range(npairs)]
        ones65 = persist.tile([65, hd], f16)

        # ---- input loads ----
        if xt_one_dma:
            nc.sync.dma_start(xt_sb[:], xt.rearrange("(ko ki) s -> ki ko s",
                                                     ki=P))
        else:
            for ko in range(kt):
                nc.sync.dma_start(xt_sb[:, ko, :], xt[ko * P:(ko + 1) * P, :])
        nc.sync.dma_start(wq_sb[:], wq.rearrange("(ko ki) m -> ki ko m", ki=P))
        nc.sync.dma_start(wk_sb[:], wk.rearrange("(ko ki) m -> ki ko m", ki=P))
        nc.sync.dma_start(wv_sb[:], wv.rearrange("(ko ki) m -> ki ko m", ki=P))
        nc.sync.dma_start(wo_sb[:], wo.rearrange("(t p) n -> p t n", p=min(P, dmc)))
        nc.vector.memset(v_sb[:, :, :, hd:hd + 1], 1.0)
        nc.vector.memset(ones65[:], 1.0)

        def proj_kq_tile(mt, which, n0, nw):
            """One [mp, nw] tile of kT (which=0) or qT (which=1) for m-tile mt."""
            mp = min(P, dmc)
            w_sb, out_sb = ((wk_sb, kt_sb), (wq_sb, qt_sb))[which]
            ps = spare.tile([mp, NC5], f32, tag="ps512", name="ps")
            for ko in range(kt):
                nc.tensor.matmul(
                    ps[:, 0:nw],
                    lhsT=w_sb[:, ko, mt * mp:(mt + 1) * mp],
                    rhs=xt_sb[:, ko, n0:n0 + nw],
                    start=(ko == 0), stop=(ko == kt - 1))
            nc.vector.tensor_copy(
                out_sb[0:mp, mt, n0:n0 + nw], ps[:, 0:nw])

        def proj_v_tile(jt):
            """v natural layout [seq, dmc] -> v_sb[:, jt, h, 0:hd]."""
            ps = spare.tile([P, dmc], f32, tag="ps512", name="ps")
            for ko in range(kt):
                nc.tensor.matmul(
                    ps[:],
                    lhsT=xt_sb[:, ko, jt * P:(jt + 1) * P],
                    rhs=wv_sb[:, ko, :],
                    start=(ko == 0), stop=(ko == kt - 1))
            nc.vector.tensor_copy(
                v_sb[:, jt, :, 0:hd],
                ps.rearrange("p (h d) -> p h d", h=nh))

        def attn_head_ih(h, ih, nm_override=None):
            if True:
                nmode = nm_override or norm_mode
                mt = h // heads_per_mtile
                hb = (h % heads_per_mtile) * hd
                pair = h // heads_per_mtile
                i0 = ih * ihw
                uacc = uaccp.tile([hd + 1, ihw], f32, tag="uacc")
                for jt in range(seqt):
                    sim = simp.tile([P, ihw], f32, tag="sim")
                    for c0, cw in chunks(ihw):
                        nc.tensor.matmul(
                            sim[:, c0:c0 + cw],
                            lhsT=kt_sb[hb:hb + hd, mt, jt * P:(jt + 1) * P],
                            rhs=qt_sb[hb:hb + hd, mt, i0 + c0:i0 + c0 + cw],
                            start=True, stop=True)
                    expt = expp.tile([P, ihw], f16, tag="expt")
                    nc.scalar.activation(expt[:], sim[:], Exp, scale=scale)
                    for c0, cw in chunks(ihw):
                        nc.tensor.matmul(
                            uacc[:, c0:c0 + cw],
                            lhsT=v_sb[:, jt, h, :],
                            rhs=expt[:, c0:c0 + cw],
                            start=(jt == 0), stop=(jt == seqt - 1))
                # normalization: r = 1/s; U_norm = U~ * broadcast(r)
                u_sb = upool.tile([hd, ihw], f16, tag="u")
                nc.vector.tensor_copy(u_sb[:], uacc[0:hd, :])
                if skip_norm:
                    nc.vector.tensor_copy(
                        upairs[pair][hb:hb + hd, i0:i0 + ihw], u_sb[:])
                    return
                srow = rows.tile([65, ihw], f32, tag="srow")
                nc.vector.tensor_copy(srow[64:65, :], uacc[hd:hd + 1, :])
                # r = 1/s. The fused custom-DVE reciprocal op returns garbage
                # on this HW path, so either ACT ln/exp ("ln") or a manual
                # Newton iteration from standard DVE ops ("dve", default —
                # keeps the critical ACT engine free for the softmax exps).
                rrow = rows.tile([65, ihw], f32, tag="rrow")
                rrow16 = rows.tile([65, ihw], f16, tag="rrow16")
                if nmode == "ln":
                    lnrow = rows.tile([65, ihw], f32, tag="lnrow")
                    nc.scalar.activation(lnrow[64:65, :], srow[64:65, :], Ln)
                    nc.scalar.activation(rrow[64:65, :], lnrow[64:65, :], Exp,
                                         scale=-1.0)
                    nc.vector.tensor_copy(rrow16[64:65, :], rrow[64:65, :])
                elif nmode == "dve":
                    i32 = mybir.dt.int32
                    s_r, u_r, t_r = (srow[64:65, :], rrow[64:65, :],
                                     None)
                    trow = rows.tile([65, ihw], f32, tag="trow")
                    t_r = trow[64:65, :]
                    # u0 = bitcast(~bits(s)) * 0.23549792   (u = -1/s approx)
                    nc.vector.tensor_scalar(t_r.bitcast(i32), s_r.bitcast(i32),
                                            -1, None,
                                            op0=mybir.AluOpType.bitwise_xor)
                    nc.vector.tensor_scalar_mul(u_r, t_r, 0.23549792)
                    # two Newton passes: u <- (s*u + c)*u, c = 2.0017324, 2.0
                    for c in (2.0017324, 2.0):
                        nc.vector.tensor_mul(t_r, s_r, u_r)
                        nc.vector.scalar_tensor_tensor(
                            u_r, t_r, float(c), u_r,
                            op0=mybir.AluOpType.add, op1=mybir.AluOpType.mult)
                    # r = -u, cast to fp16
                    nc.vector.tensor_scalar_mul(rrow16[64:65, :], u_r, -1.0)
                elif nmode == "copy":  # timing-only bisect: wrong math
                    nc.vector.tensor_copy(rrow16[64:65, :], srow[64:65, :])
                else:
                    raise ValueError(nmode)
                for c0, cw in chunks(ihw):
                    rps = spare.tile([hd, NC5], f32, tag="ps512")
                    nc.tensor.matmul(
                        rps[:, 0:cw],
                        lhsT=ones65[64:65, :],
                        rhs=rrow16[64:65, c0:c0 + cw],
                        start=True, stop=True)
                    nc.vector.tensor_mul(
                        upairs[pair][hb:hb + hd, i0 + c0:i0 + c0 + cw],
                        u_sb[:, c0:c0 + cw], rps[:, 0:cw])

        def final_proj(ms=None):
            mp = min(P, dmc)
            for m in (range(seqt) if ms is None else ms):
                if y_pair_dma:
                    yt = ysb.tile([P, dout], f32, tag="yt")
                for n0, nw in chunks(dout):
                    yps = spare.tile([P, NC5], f32, tag="ps512")
                    for p in range(npairs):
                        nc.tensor.matmul(
                            yps[:, 0:nw],
                            lhsT=upairs[p][0:mp, m * P:(m + 1) * P],
                            rhs=wo_sb[0:mp, p, n0:n0 + nw],
                            start=(p == 0), stop=(p == npairs - 1))
                    if y_pair_dma:
                        nc.vector.tensor_copy(yt[:, n0:n0 + nw], yps[:, 0:nw])
                    else:
                        yt = ysb.tile([P, NC5], f32, tag="yt")
                        nc.vector.tensor_copy(yt[:, 0:nw], yps[:, 0:nw])
                        nc.sync.dma_start(
                            y[m * P:(m + 1) * P, n0:n0 + nw], yt[:, 0:nw])
                if y_pair_dma:
                    nc.sync.dma_start(y[m * P:(m + 1) * P, :], yt[:])

        # Emission schedule: per-engine instruction order is static after
        # scheduling, so projection granules are threaded between attention
        # (h, ih) phases — each phase's inputs emitted one phase ahead; the
        # ACT-paced attention then hides the remaining projection PE work.
        attn_phases = [(h, ih) for h in range(nh) for ih in range(n_ih)]

        def phase_needs(idx):
            # granules that must be emitted before attention phase idx;
            # every phase's j-loop consumes ALL v tiles, so v has deadline 0.
            if idx >= len(attn_phases):
                return []
            h, ih = attn_phases[idx]
            mt = h // heads_per_mtile
            need = [("k", mt, n0, nw) for n0, nw in chunks(seq)]
            need += [("q", mt, n0, nw) for n0, nw in chunks(seq)
                     if n0 < (ih + 1) * ihw and n0 + nw > ih * ihw]
            if idx == 0:
                need += [("v", jt) for jt in range(seqt)]
            return need

        emitted = set()

        def emit_granules(needs):
            for g in needs:
                if g in emitted:
                    continue
                emitted.add(g)
                if g[0] == "v":
                    proj_v_tile(g[1])
                else:
                    which = 0 if g[0] == "k" else 1
                    proj_kq_tile(g[1], which, g[2], g[3])

        all_granules = []
        for idx in range(len(attn_phases)):
            for g in phase_needs(idx):
                if g not in all_granules:
                    all_granules.append(g)

        if phases == 'proj':
            emit_granules(all_granules)
        else:
            emit_granules(phase_needs(0))
            # deadline-ordered backlog, spread evenly across early boundaries
            backlog = [g for g in all_granules if g not in emitted]
            nb = max(1, len(attn_phases) - 2)
            share = -(-len(backlog) // nb)
            last = len(attn_phases) - 1
            for idx, (h, ih) in enumerate(attn_phases):
                attn_head_ih(h, ih, nm_override="ln" if idx == last else None)
                emit_granules(phase_needs(idx + 1))
                take = [g for g in backlog if g not in emitted][:share]
                emit_granules(take)
                if phases == 'all' and idx == last - 1 and n_ih > 1:
                    # final-proj m-tiles whose i-range completes at the
                    # second-to-last phase overlap the last phase's attention
                    lh, lih = attn_phases[last]
                    done_ih = [p_ih for p_ih in range(n_ih) if p_ih != lih]
                    ms = [m for m in range(seqt)
                          if (m * P) // ihw in done_ih]
                    final_proj(ms)
            if phases == 'all':
                lh, lih = attn_phases[last]
                if n_ih > 1:
                    final_proj([m for m in range(seqt)
                                if (m * P) // ihw == lih])
                else:
                    final_proj()

    nc.compile()
    return nc


_NC_CACHE = {}


def _get_nc():
    if "nc" not in _NC_CACHE:
        _NC_CACHE["nc"] = build_nc()
    return _NC_CACHE["nc"]


def _prep_core_inputs(x, Wq, Wkv, Wo):
    """Host-side shard + layout prep: per-core fp16 slices."""
    f16 = np.float16
    in_maps = []
    for c in range(N_CORES):
        b, g = c // 2, c % 2
        s = slice(g * DMC, (g + 1) * DMC)
        in_maps.append({
            "xt": np.ascontiguousarray(x[b].T).astype(f16),
            "wq": np.ascontiguousarray(Wq[:, s]).astype(f16),
            "wk": np.ascontiguousarray(Wkv[:, g * DMC:(g + 1) * DMC]).astype(f16),
            "wv": np.ascontiguousarray(
                Wkv[:, DIM_MODEL + g * DMC:DIM_MODEL + (g + 1) * DMC]).astype(f16),
            "wo": np.ascontiguousarray(Wo[s, :]).astype(f16),
        })
    return in_maps


def kernel(x, Wq, Wkv, Wo, bo):
    from concourse import bass_utils

    x = np.asarray(x, dtype=np.float32)
    Wq = np.asarray(Wq, dtype=np.float32)
    Wkv = np.asarray(Wkv, dtype=np.float32)
    Wo = np.asarray(Wo, dtype=np.float32)
    bo = np.asarray(bo, dtype=np.float32)

    nc = _get_nc()
    in_maps = _prep_core_inputs(x, Wq, Wkv, Wo)
    res = bass_utils.run_bass_kernel_spmd(nc, in_maps,
                                          core_ids=list(range(N_CORES)))
    out = np.empty((B, N, QDIM), dtype=np.float32)
    for b in range(B):
        out[b] = res.results[2 * b]["y"] + res.results[2 * b + 1]["y"] + bo
    return out


# revision 20
# speedup vs baseline: 1.1383x; 1.0968x over previous
"""Multi-head self-attention on 8 Trainium2 NeuronCores.

Problem: x:(4,2048,1024) fp32; q = x@Wq, kv = x@Wkv (k,v split), 8 heads of
dim 64, softmax(q k^T / 8) v, concat heads, @Wo + bo -> (4,2048,1024).

Sharding: core c handles batch b=c//2 and head group g=c%2 (4 of 8 heads).
Each core computes its batch's projections restricted to its 4 heads, full
attention for those heads, and a partial output projection y_c = U_norm @ Wo_g.
Host gathers: out[b] = y_{2b} + y_{2b+1} + bo  (the "all-reduce" of the
tensor-parallel head split, done at unshard time).

Device algorithm (per core), all matmul operands fp16, PSUM accumulate fp32:
  - host supplies xT = x[b].T so the contraction dim (QDIM) is the partition
    axis; projections compute qT/kT (head_dim-major) and v (seq-major) tiles.
  - attention per head, per i-half (1024 q rows), per j-tile (128 k rows):
      simT[j,i] = kT_h(j)^T-tile @ qT_h        (PE, K=64)
      expT = exp(SCALE*simT)                   (ACT, reads PSUM directly)
      U~[d,i] += [v_h | 1]^T @ expT            (PE, K=128; row 64 = softmax sum)
    then normalization: r = 1/s via fast-reciprocal (DVE), R = ones x r
    broadcast (PE K=1 matmul), U_norm = U~ * R (DVE).
  - y[m,:] = U_norm_pairs^T @ Wo_g (K=128 per head pair), DVE drain, DMA out.
"""

import numpy as np

# ---- problem constants (hardcoded per the harness contract) ----
B, N, QDIM = 4, 2048, 1024
HEADS, DIM_MODEL = 8, 512
HEAD_DIM = DIM_MODEL // HEADS  # 64
SCALE = HEAD_DIM ** -0.5  # 0.125
N_CORES = 8
HEADS_PER_CORE = HEADS // 2  # 4 (head-group split across 2 cores per batch)
DMC = HEADS_PER_CORE * HEAD_DIM  # 256 per-core model dim slice


def build_nc(seq=N, qd=QDIM, nh=HEADS_PER_CORE, hd=HEAD_DIM, dout=QDIM,
             scale=SCALE, ihw=1024, skip_norm=False, norm_mode='dve',
             expp_bufs=3, upool_bufs=2, rows_bufs=2, ysb_bufs=3,
             phases='all', simp_bufs=2, uaccp_bufs=1, spare_bufs=2,
             xt_one_dma=True, y_pair_dma=True):
    """Build the per-core Bass program (same program on all 8 cores)."""
    from contextlib import ExitStack

    import concourse.bass as bass
    import concourse.tile as tile
    from concourse import bacc, mybir

    P = 128
    NC5 = 512  # psum bank width in fp32
    f16 = mybir.dt.float16
    f32 = mybir.dt.float32
    Exp = mybir.ActivationFunctionType.Exp
    Ln = mybir.ActivationFunctionType.Ln

    dmc = nh * hd                 # per-core projected dim (256)
    kt = qd // P                  # contraction tiles over QDIM (8)
    seqt = seq // P               # seq tiles (16)
    mtiles = max(1, dmc // P)     # qT/kT partition tiles (2)
    heads_per_mtile = nh // mtiles
    ihw = min(ihw, seq)           # i-half width
    n_ih = seq // ihw
    npairs = mtiles               # head pairs stacked for final proj (2)

    def chunks(total, w=NC5):
        c0 = 0
        while c0 < total:
            yield c0, min(w, total - c0)
            c0 += w

    nc = bacc.Bacc("TRN2", target_bir_lowering=False, debug=False,
                   num_devices=N_CORES)

    xt = nc.dram_tensor("xt", (qd, seq), f16, kind="ExternalInput").ap()
    wq = nc.dram_tensor("wq", (qd, dmc), f16, kind="ExternalInput").ap()
    wk = nc.dram_tensor("wk", (qd, dmc), f16, kind="ExternalInput").ap()
    wv = nc.dram_tensor("wv", (qd, dmc), f16, kind="ExternalInput").ap()
    wo = nc.dram_tensor("wo", (dmc, dout), f16, kind="ExternalInput").ap()
    y = nc.dram_tensor("y", (seq, dout), f32, kind="ExternalOutput").ap()

    with tile.TileContext(nc) as tc, ExitStack() as ctx:
        # ---- SBUF pools ----
        persist = ctx.enter_context(tc.tile_pool(name="persist", bufs=1))
        expp = ctx.enter_context(tc.tile_pool(name="expp", bufs=expp_bufs))
        upool = ctx.enter_context(tc.tile_pool(name="upool", bufs=upool_bufs))
        rows = ctx.enter_context(tc.tile_pool(name="rows", bufs=rows_bufs))
        ysb = ctx.enter_context(tc.tile_pool(name="ysb", bufs=ysb_bufs))
        # ---- PSUM pools (8 banks total: 2 spare + 4 sim + 2 uacc) ----
        spare = ctx.enter_context(tc.tile_pool(name="spare", bufs=spare_bufs, space="PSUM"))
        simp = ctx.enter_context(tc.tile_pool(name="simp", bufs=simp_bufs, space="PSUM"))
        uaccp = ctx.enter_context(tc.tile_pool(name="uaccp", bufs=uaccp_bufs, space="PSUM"))

        # ---- persistent SBUF tensors ----
        xt_sb = persist.tile([P, kt, seq], f16)
        wq_sb = persist.tile([P, kt, dmc], f16)
        wk_sb = persist.tile([P, kt, dmc], f16)
        wv_sb = persist.tile([P, kt, dmc], f16)
        wo_sb = persist.tile([min(P, dmc), npairs, dout], f16)
        v_sb = persist.tile([P, seqt, nh, hd + 1], f16)
        qt_sb = persist.tile([min(P, dmc), mtiles, seq], f16)
        kt_sb = persist.tile([min(P, dmc), mtiles, seq], f16)
        upairs = [persist.tile([min(P, dmc), seq], f16, name=f"upair{p}")
                  for p in range(npairs)]
        ones65 = persist.tile([65, hd], f16)

        # ---- input loads ----
        if xt_one_dma:
            nc.sync.dma_start(xt_sb[:], xt.rearrange("(ko ki) s -> ki ko s",
                                                     ki=P))
        else:
            for ko in range(kt):
                nc.sync.dma_start(xt_sb[:, ko, :], xt[ko * P:(ko + 1) * P, :])
        nc.sync.dma_start(wq_sb[:], wq.rearrange("(ko ki) m -> ki ko m", ki=P))
        nc.sync.dma_start(wk_sb[:], wk.rearrange("(ko ki) m -> ki ko m", ki=P))
        nc.sync.dma_start(wv_sb[:], wv.rearrange("(ko ki) m -> ki ko m", ki=P))
        nc.sync.dma_start(wo_sb[:], wo.rearrange("(t p) n -> p t n", p=min(P, dmc)))
        nc.vector.memset(v_sb[:, :, :, hd:hd + 1], 1.0)
        nc.vector.memset(ones65[:], 1.0)

        def proj_kq_tile(mt, which, n0, nw):
            """One [mp, nw] tile of kT (which=0) or qT (which=1) for m-tile mt."""
            mp = min(P, dmc)
            w_sb, out_sb = ((wk_sb, kt_sb), (wq_sb, qt_sb))[which]
            ps = spare.tile([mp, NC5], f32, tag="ps512", name="ps")
            for ko in range(kt):
                nc.tensor.matmul(
                    ps[:, 0:nw],
                    lhsT=w_sb[:, ko, mt * mp:(mt + 1) * mp],
                    rhs=xt_sb[:, ko, n0:n0 + nw],
                    start=(ko == 0), stop=(ko == kt - 1))
            nc.vector.tensor_copy(
                out_sb[0:mp, mt, n0:n0 + nw], ps[:, 0:nw])

        def proj_v_tile(jt):
            """v natural layout [seq, dmc] -> v_sb[:, jt, h, 0:hd]."""
            ps = spare.tile([P, dmc], f32, tag="ps512", name="ps")
            for ko in range(kt):
                nc.tensor.matmul(
                    ps[:],
                    lhsT=xt_sb[:, ko, jt * P:(jt + 1) * P],
                    rhs=wv_sb[:, ko, :],
                    start=(ko == 0), stop=(ko == kt - 1))
            nc.vector.tensor_copy(
                v_sb[:, jt, :, 0:hd],
                ps.rearrange("p (h d) -> p h d", h=nh))

        def attn_head_ih(h, ih, nm_override=None, pre_norm_cb=None):
            if True:
                nmode = nm_override or norm_mode
                mt = h // heads_per_mtile
                hb = (h % heads_per_mtile) * hd
                pair = h // heads_per_mtile
                i0 = ih * ihw
                uacc = uaccp.tile([hd + 1, ihw], f32, tag="uacc")
                for jt in range(seqt):
                    sim = simp.tile([P, ihw], f32, tag="sim")
                    for c0, cw in chunks(ihw):
                        nc.tensor.matmul(
                            sim[:, c0:c0 + cw],
                            lhsT=kt_sb[hb:hb + hd, mt, jt * P:(jt + 1) * P],
                            rhs=qt_sb[hb:hb + hd, mt, i0 + c0:i0 + c0 + cw],
                            start=True, stop=True)
                    expt = expp.tile([P, ihw], f16, tag="expt")
                    nc.scalar.activation(expt[:], sim[:], Exp, scale=scale)
                    for c0, cw in chunks(ihw):
                        nc.tensor.matmul(
                            uacc[:, c0:c0 + cw],
                            lhsT=v_sb[:, jt, h, :],
                            rhs=expt[:, c0:c0 + cw],
                            start=(jt == 0), stop=(jt == seqt - 1))
                if pre_norm_cb is not None:
                    # emit next phase's projection granules here so their DVE
                    # drains are ordered BEFORE this phase's norm chain
                    pre_norm_cb()
                # normalization: r = 1/s; U_norm = U~ * broadcast(r)
                u_sb = upool.tile([hd, ihw], f16, tag="u")
                nc.vector.tensor_copy(u_sb[:], uacc[0:hd, :])
                if skip_norm:
                    nc.vector.tensor_copy(
                        upairs[pair][hb:hb + hd, i0:i0 + ihw], u_sb[:])
                    return
                srow = rows.tile([65, ihw], f32, tag="srow")
                nc.vector.tensor_copy(srow[64:65, :], uacc[hd:hd + 1, :])
                # r = 1/s. The fused custom-DVE reciprocal op returns garbage
                # on this HW path, so either ACT ln/exp ("ln") or a manual
                # Newton iteration from standard DVE ops ("dve", default —
                # keeps the critical ACT engine free for the softmax exps).
                rrow = rows.tile([65, ihw], f32, tag="rrow")
                rrow16 = rows.tile([65, ihw], f16, tag="rrow16")
                if nmode == "ln":
                    lnrow = rows.tile([65, ihw], f32, tag="lnrow")
                    nc.scalar.activation(lnrow[64:65, :], srow[64:65, :], Ln)
                    nc.scalar.activation(rrow[64:65, :], lnrow[64:65, :], Exp,
                                         scale=-1.0)
                    nc.vector.tensor_copy(rrow16[64:65, :], rrow[64:65, :])
                elif nmode == "dve":
                    i32 = mybir.dt.int32
                    s_r, u_r, t_r = (srow[64:65, :], rrow[64:65, :],
                                     None)
                    trow = rows.tile([65, ihw], f32, tag="trow")
                    t_r = trow[64:65, :]
                    # u0 = bitcast(~bits(s)) * 0.23549792   (u = -1/s approx)
                    nc.vector.tensor_scalar(t_r.bitcast(i32), s_r.bitcast(i32),
                                            -1, None,
                                            op0=mybir.AluOpType.bitwise_xor)
                    nc.vector.tensor_scalar_mul(u_r, t_r, 0.23549792)
                    # two Newton passes: u <- (s*u + c)*u, c = 2.0017324, 2.0
                    for c in (2.0017324, 2.0):
                        nc.vector.tensor_mul(t_r, s_r, u_r)
                        nc.vector.scalar_tensor_tensor(
                            u_r, t_r, float(c), u_r,
                            op0=mybir.AluOpType.add, op1=mybir.AluOpType.mult)
                    # r = -u, cast to fp16
                    nc.vector.tensor_scalar_mul(rrow16[64:65, :], u_r, -1.0)
                elif nmode == "copy":  # timing-only bisect: wrong math
                    nc.vector.tensor_copy(rrow16[64:65, :], srow[64:65, :])
                else:
                    raise ValueError(nmode)
                for c0, cw in chunks(ihw):
                    rps = spare.tile([hd, NC5], f32, tag="ps512")
                    nc.tensor.matmul(
                        rps[:, 0:cw],
                        lhsT=ones65[64:65, :],
                        rhs=rrow16[64:65, c0:c0 + cw],
                        start=True, stop=True)
                    nc.vector.tensor_mul(
                        upairs[pair][hb:hb + hd, i0 + c0:i0 + c0 + cw],
                        u_sb[:, c0:c0 + cw], rps[:, 0:cw])

        def final_proj(ms=None):
            mp = min(P, dmc)
            for m in (range(seqt) if ms is None else ms):
                if y_pair_dma:
                    yt = ysb.tile([P, dout], f32, tag="yt")
                for n0, nw in chunks(dout):
                    yps = spare.tile([P, NC5], f32, tag="ps512")
                    for p in range(npairs):
                        nc.tensor.matmul(
                            yps[:, 0:nw],
                            lhsT=upairs[p][0:mp, m * P:(m + 1) * P],
                            rhs=wo_sb[0:mp, p, n0:n0 + nw],
                            start=(p == 0), stop=(p == npairs - 1))
                    if y_pair_dma:
                        nc.vector.tensor_copy(yt[:, n0:n0 + nw], yps[:, 0:nw])
                    else:
                        yt = ysb.tile([P, NC5], f32, tag="yt")
                        nc.vector.tensor_copy(yt[:, 0:nw], yps[:, 0:nw])
                        nc.sync.dma_start(
                            y[m * P:(m + 1) * P, n0:n0 + nw], yt[:, 0:nw])
                if y_pair_dma:
                    nc.sync.dma_start(y[m * P:(m + 1) * P, :], yt[:])

        # Emission schedule: per-engine instruction order is static after
        # scheduling, so projection granules are threaded between attention
        # (h, ih) phases — each phase's inputs emitted one phase ahead; the
        # ACT-paced attention then hides the remaining projection PE work.
        attn_phases = [(h, ih) for h in range(nh) for ih in range(n_ih)]

        def phase_needs(idx):
            # granules that must be emitted before attention phase idx;
            # every phase's j-loop consumes ALL v tiles, so v has deadline 0.
            if idx >= len(attn_phases):
                return []
            h, ih = attn_phases[idx]
            mt = h // heads_per_mtile
            need = [("k", mt, n0, nw) for n0, nw in chunks(seq)]
            need += [("q", mt, n0, nw) for n0, nw in chunks(seq)
                     if n0 < (ih + 1) * ihw and n0 + nw > ih * ihw]
            if idx == 0:
                need += [("v", jt) for jt in range(seqt)]
            return need

        emitted = set()

        def emit_granules(needs):
            for g in needs:
                if g in emitted:
                    continue
                emitted.add(g)
                if g[0] == "v":
                    proj_v_tile(g[1])
                else:
                    which = 0 if g[0] == "k" else 1
                    proj_kq_tile(g[1], which, g[2], g[3])

        all_granules = []
        for idx in range(len(attn_phases)):
            for g in phase_needs(idx):
                if g not in all_granules:
                    all_granules.append(g)

        if phases == 'proj':
            emit_granules(all_granules)
        else:
            emit_granules(phase_needs(0))
            # deadline-ordered backlog, spread evenly across early boundaries
            backlog = [g for g in all_granules if g not in emitted]
            nb = max(1, len(attn_phases) - 2)
            share = -(-len(backlog) // nb)
            last = len(attn_phases) - 1
            for idx, (h, ih) in enumerate(attn_phases):
                def _cb(idx=idx):
                    emit_granules(phase_needs(idx + 1))
                    take = [g for g in backlog if g not in emitted][:share]
                    emit_granules(take)
                attn_head_ih(h, ih, nm_override="ln" if idx == last else None,
                             pre_norm_cb=_cb)
                if phases == 'all' and idx == last - 1 and n_ih > 1:
                    # final-proj m-tiles whose i-range completes at the
                    # second-to-last phase overlap the last phase's attention
                    lh, lih = attn_phases[last]
                    done_ih = [p_ih for p_ih in range(n_ih) if p_ih != lih]
                    ms = [m for m in range(seqt)
                          if (m * P) // ihw in done_ih]
                    final_proj(ms)
            if phases == 'all':
                lh, lih = attn_phases[last]
                if n_ih > 1:
                    final_proj([m for m in range(seqt)
                                if (m * P) // ihw == lih])
                else:
                    final_proj()

    nc.compile()
    return nc


_NC_CACHE = {}


def _get_nc():
    if "nc" not in _NC_CACHE:
        _NC_CACHE["nc"] = build_nc()
    return _NC_CACHE["nc"]


def _prep_core_inputs(x, Wq, Wkv, Wo):
    """Host-side shard + layout prep: per-core fp16 slices."""
    f16 = np.float16
    in_maps = []
    for c in range(N_CORES):
        b, g = c // 2, c % 2
        s = slice(g * DMC, (g + 1) * DMC)
        in_maps.append({
            "xt": np.ascontiguousarray(x[b].T).astype(f16),
            "wq": np.ascontiguousarray(Wq[:, s]).astype(f16),
            "wk": np.ascontiguousarray(Wkv[:, g * DMC:(g + 1) * DMC]).astype(f16),
            "wv": np.ascontiguousarray(
                Wkv[:, DIM_MODEL + g * DMC:DIM_MODEL + (g + 1) * DMC]).astype(f16),
            "wo": np.ascontiguousarray(Wo[s, :]).astype(f16),
        })
    return in_maps


def kernel(x, Wq, Wkv, Wo, bo):
    from concourse import bass_utils

    x = np.asarray(x, dtype=np.float32)
    Wq = np.asarray(Wq, dtype=np.float32)
    Wkv = np.asarray(Wkv, dtype=np.float32)
    Wo = np.asarray(Wo, dtype=np.float32)
    bo = np.asarray(bo, dtype=np.float32)

    nc = _get_nc()
    in_maps = _prep_core_inputs(x, Wq, Wkv, Wo)
    res = bass_utils.run_bass_kernel_spmd(nc, in_maps,
                                          core_ids=list(range(N_CORES)))
    out = np.empty((B, N, QDIM), dtype=np.float32)
    for b in range(B):
        out[b] = res.results[2 * b]["y"] + res.results[2 * b + 1]["y"] + bo
    return out


# revision 24
# speedup vs baseline: 1.1451x; 1.0060x over previous
"""Multi-head self-attention on 8 Trainium2 NeuronCores.

Problem: x:(4,2048,1024) fp32; q = x@Wq, kv = x@Wkv (k,v split), 8 heads of
dim 64, softmax(q k^T / 8) v, concat heads, @Wo + bo -> (4,2048,1024).

Sharding: core c handles batch b=c//2 and head group g=c%2 (4 of 8 heads).
Each core computes its batch's projections restricted to its 4 heads, full
attention for those heads, and a partial output projection y_c = U_norm @ Wo_g.
Host gathers: out[b] = y_{2b} + y_{2b+1} + bo  (the "all-reduce" of the
tensor-parallel head split, done at unshard time).

Device algorithm (per core), all matmul operands fp16, PSUM accumulate fp32:
  - host supplies xT = x[b].T so the contraction dim (QDIM) is the partition
    axis; projections compute qT/kT (head_dim-major) and v (seq-major) tiles.
  - attention per head, per i-half (1024 q rows), per j-tile (128 k rows):
      simT[j,i] = kT_h(j)^T-tile @ qT_h        (PE, K=64)
      expT = exp(SCALE*simT)                   (ACT, reads PSUM directly)
      U~[d,i] += [v_h | 1]^T @ expT            (PE, K=128; row 64 = softmax sum)
    then normalization: r = 1/s via fast-reciprocal (DVE), R = ones x r
    broadcast (PE K=1 matmul), U_norm = U~ * R (DVE).
  - y[m,:] = U_norm_pairs^T @ Wo_g (K=128 per head pair), DVE drain, DMA out.
"""

import numpy as np

# ---- problem constants (hardcoded per the harness contract) ----
B, N, QDIM = 4, 2048, 1024
HEADS, DIM_MODEL = 8, 512
HEAD_DIM = DIM_MODEL // HEADS  # 64
SCALE = HEAD_DIM ** -0.5  # 0.125
N_CORES = 8
HEADS_PER_CORE = HEADS // 2  # 4 (head-group split across 2 cores per batch)
DMC = HEADS_PER_CORE * HEAD_DIM  # 256 per-core model dim slice


def build_nc(seq=N, qd=QDIM, nh=HEADS_PER_CORE, hd=HEAD_DIM, dout=QDIM,
             scale=SCALE, ihw=1024, skip_norm=False, norm_mode='dve',
             expp_bufs=3, upool_bufs=2, rows_bufs=2, ysb_bufs=3,
             phases='all', simp_bufs=2, uaccp_bufs=1, spare_bufs=2,
             xt_one_dma=True, y_pair_dma=True):
    """Build the per-core Bass program (same program on all 8 cores)."""
    from contextlib import ExitStack

    import concourse.bass as bass
    import concourse.tile as tile
    from concourse import bacc, mybir

    P = 128
    NC5 = 512  # psum bank width in fp32
    f16 = mybir.dt.float16
    f32 = mybir.dt.float32
    Exp = mybir.ActivationFunctionType.Exp
    Ln = mybir.ActivationFunctionType.Ln

    dmc = nh * hd                 # per-core projected dim (256)
    kt = qd // P                  # contraction tiles over QDIM (8)
    seqt = seq // P               # seq tiles (16)
    mtiles = max(1, dmc // P)     # qT/kT partition tiles (2)
    heads_per_mtile = nh // mtiles
    ihw = min(ihw, seq)           # i-half width
    n_ih = seq // ihw
    npairs = mtiles               # head pairs stacked for final proj (2)

    def chunks(total, w=NC5):
        c0 = 0
        while c0 < total:
            yield c0, min(w, total - c0)
            c0 += w

    nc = bacc.Bacc("TRN2", target_bir_lowering=False, debug=False,
                   num_devices=N_CORES)

    xt = nc.dram_tensor("xt", (qd, seq), f16, kind="ExternalInput").ap()
    wq = nc.dram_tensor("wq", (qd, dmc), f16, kind="ExternalInput").ap()
    wk = nc.dram_tensor("wk", (qd, dmc), f16, kind="ExternalInput").ap()
    wv = nc.dram_tensor("wv", (qd, dmc), f16, kind="ExternalInput").ap()
    wo = nc.dram_tensor("wo", (dmc, dout), f16, kind="ExternalInput").ap()
    y = nc.dram_tensor("y", (seq, dout), f32, kind="ExternalOutput").ap()

    with tile.TileContext(nc) as tc, ExitStack() as ctx:
        # ---- SBUF pools ----
        persist = ctx.enter_context(tc.tile_pool(name="persist", bufs=1))
        expp = ctx.enter_context(tc.tile_pool(name="expp", bufs=expp_bufs))
        upool = ctx.enter_context(tc.tile_pool(name="upool", bufs=upool_bufs))
        rows = ctx.enter_context(tc.tile_pool(name="rows", bufs=rows_bufs))
        ysb = ctx.enter_context(tc.tile_pool(name="ysb", bufs=ysb_bufs))
        # ---- PSUM pools (8 banks total: 2 spare + 4 sim + 2 uacc) ----
        spare = ctx.enter_context(tc.tile_pool(name="spare", bufs=spare_bufs, space="PSUM"))
        simp = ctx.enter_context(tc.tile_pool(name="simp", bufs=simp_bufs, space="PSUM"))
        uaccp = ctx.enter_context(tc.tile_pool(name="uaccp", bufs=uaccp_bufs, space="PSUM"))

        # ---- persistent SBUF tensors ----
        xt_sb = persist.tile([P, kt, seq], f16)
        wq_sb = persist.tile([P, kt, dmc], f16)
        wk_sb = persist.tile([P, kt, dmc], f16)
        wv_sb = persist.tile([P, kt, dmc], f16)
        wo_sb = persist.tile([min(P, dmc), npairs, dout], f16)
        v_sb = persist.tile([P, seqt, nh, hd + 1], f16)
        qt_sb = persist.tile([min(P, dmc), mtiles, seq], f16)
        kt_sb = persist.tile([min(P, dmc), mtiles, seq], f16)
        upairs = [persist.tile([min(P, dmc), seq], f16, name=f"upair{p}")
                  for p in range(npairs)]
        ones65 = persist.tile([65, hd], f16)

        # ---- input loads ----
        if xt_one_dma:
            nc.sync.dma_start(xt_sb[:], xt.rearrange("(ko ki) s -> ki ko s",
                                                     ki=P))
        else:
            for ko in range(kt):
                nc.sync.dma_start(xt_sb[:, ko, :], xt[ko * P:(ko + 1) * P, :])
        nc.sync.dma_start(wk_sb[:], wk.rearrange("(ko ki) m -> ki ko m", ki=P))
        nc.sync.dma_start(wq_sb[:], wq.rearrange("(ko ki) m -> ki ko m", ki=P))
        nc.sync.dma_start(wv_sb[:], wv.rearrange("(ko ki) m -> ki ko m", ki=P))
        nc.sync.dma_start(wo_sb[:], wo.rearrange("(t p) n -> p t n", p=min(P, dmc)))
        nc.vector.memset(v_sb[:, :, :, hd:hd + 1], 1.0)
        nc.vector.memset(ones65[:], 1.0)

        def proj_kq_tile(mt, which, n0, nw):
            """One [mp, nw] tile of kT (which=0) or qT (which=1) for m-tile mt."""
            mp = min(P, dmc)
            w_sb, out_sb = ((wk_sb, kt_sb), (wq_sb, qt_sb))[which]
            ps = spare.tile([mp, NC5], f32, tag="ps512", name="ps")
            for ko in range(kt):
                nc.tensor.matmul(
                    ps[:, 0:nw],
                    lhsT=w_sb[:, ko, mt * mp:(mt + 1) * mp],
                    rhs=xt_sb[:, ko, n0:n0 + nw],
                    start=(ko == 0), stop=(ko == kt - 1))
            nc.vector.tensor_copy(
                out_sb[0:mp, mt, n0:n0 + nw], ps[:, 0:nw])

        def proj_v_tile(jt):
            """v natural layout [seq, dmc] -> v_sb[:, jt, h, 0:hd]."""
            ps = spare.tile([P, dmc], f32, tag="ps512", name="ps")
            for ko in range(kt):
                nc.tensor.matmul(
                    ps[:],
                    lhsT=xt_sb[:, ko, jt * P:(jt + 1) * P],
                    rhs=wv_sb[:, ko, :],
                    start=(ko == 0), stop=(ko == kt - 1))
            nc.vector.tensor_copy(
                v_sb[:, jt, :, 0:hd],
                ps.rearrange("p (h d) -> p h d", h=nh))

        def attn_head_ih(h, ih, nm_override=None, pre_norm_cb=None):
            if True:
                nmode = nm_override or norm_mode
                mt = h // heads_per_mtile
                hb = (h % heads_per_mtile) * hd
                pair = h // heads_per_mtile
                i0 = ih * ihw
                uacc = uaccp.tile([hd + 1, ihw], f32, tag="uacc")
                for jt in range(seqt):
                    sim = simp.tile([P, ihw], f32, tag="sim")
                    for c0, cw in chunks(ihw):
                        nc.tensor.matmul(
                            sim[:, c0:c0 + cw],
                            lhsT=kt_sb[hb:hb + hd, mt, jt * P:(jt + 1) * P],
                            rhs=qt_sb[hb:hb + hd, mt, i0 + c0:i0 + c0 + cw],
                            start=True, stop=True)
                    expt = expp.tile([P, ihw], f16, tag="expt")
                    nc.scalar.activation(expt[:], sim[:], Exp, scale=scale)
                    for c0, cw in chunks(ihw):
                        nc.tensor.matmul(
                            uacc[:, c0:c0 + cw],
                            lhsT=v_sb[:, jt, h, :],
                            rhs=expt[:, c0:c0 + cw],
                            start=(jt == 0), stop=(jt == seqt - 1))
                if pre_norm_cb is not None:
                    # emit next phase's projection granules here so their DVE
                    # drains are ordered BEFORE this phase's norm chain
                    pre_norm_cb()
                # normalization: r = 1/s; U_norm = U~ * broadcast(r)
                u_sb = upool.tile([hd, ihw], f16, tag="u")
                nc.vector.tensor_copy(u_sb[:], uacc[0:hd, :])
                if skip_norm:
                    nc.vector.tensor_copy(
                        upairs[pair][hb:hb + hd, i0:i0 + ihw], u_sb[:])
                    return
                srow = rows.tile([65, ihw], f32, tag="srow")
                nc.vector.tensor_copy(srow[64:65, :], uacc[hd:hd + 1, :])
                # r = 1/s. The fused custom-DVE reciprocal op returns garbage
                # on this HW path, so either ACT ln/exp ("ln") or a manual
                # Newton iteration from standard DVE ops ("dve", default —
                # keeps the critical ACT engine free for the softmax exps).
                rrow = rows.tile([65, ihw], f32, tag="rrow")
                rrow16 = rows.tile([65, ihw], f16, tag="rrow16")
                if nmode == "ln":
                    lnrow = rows.tile([65, ihw], f32, tag="lnrow")
                    nc.scalar.activation(lnrow[64:65, :], srow[64:65, :], Ln)
                    nc.scalar.activation(rrow[64:65, :], lnrow[64:65, :], Exp,
                                         scale=-1.0)
                    nc.vector.tensor_copy(rrow16[64:65, :], rrow[64:65, :])
                elif nmode == "dve":
                    i32 = mybir.dt.int32
                    s_r, u_r, t_r = (srow[64:65, :], rrow[64:65, :],
                                     None)
                    trow = rows.tile([65, ihw], f32, tag="trow")
                    t_r = trow[64:65, :]
                    # u0 = bitcast(~bits(s)) * 0.23549792   (u = -1/s approx)
                    nc.vector.tensor_scalar(t_r.bitcast(i32), s_r.bitcast(i32),
                                            -1, None,
                                            op0=mybir.AluOpType.bitwise_xor)
                    nc.vector.tensor_scalar_mul(u_r, t_r, 0.23549792)
                    # two Newton passes: u <- (s*u + c)*u, c = 2.0017324, 2.0
                    for c in (2.0017324, 2.0):
                        nc.vector.tensor_mul(t_r, s_r, u_r)
                        nc.vector.scalar_tensor_tensor(
                            u_r, t_r, float(c), u_r,
                            op0=mybir.AluOpType.add, op1=mybir.AluOpType.mult)
                    # r = -u, cast to fp16
                    nc.vector.tensor_scalar_mul(rrow16[64:65, :], u_r, -1.0)
                elif nmode == "copy":  # timing-only bisect: wrong math
                    nc.vector.tensor_copy(rrow16[64:65, :], srow[64:65, :])
                else:
                    raise ValueError(nmode)
                for c0, cw in chunks(ihw):
                    rps = spare.tile([hd, NC5], f32, tag="ps512")
                    nc.tensor.matmul(
                        rps[:, 0:cw],
                        lhsT=ones65[64:65, :],
                        rhs=rrow16[64:65, c0:c0 + cw],
                        start=True, stop=True)
                    nc.vector.tensor_mul(
                        upairs[pair][hb:hb + hd, i0 + c0:i0 + c0 + cw],
                        u_sb[:, c0:c0 + cw], rps[:, 0:cw])

        def final_proj(ms=None):
            mp = min(P, dmc)
            for m in (range(seqt) if ms is None else ms):
                if y_pair_dma:
                    yt = ysb.tile([P, dout], f32, tag="yt")
                for n0, nw in chunks(dout):
                    yps = spare.tile([P, NC5], f32, tag="ps512")
                    for p in range(npairs):
                        nc.tensor.matmul(
                            yps[:, 0:nw],
                            lhsT=upairs[p][0:mp, m * P:(m + 1) * P],
                            rhs=wo_sb[0:mp, p, n0:n0 + nw],
                            start=(p == 0), stop=(p == npairs - 1))
                    if y_pair_dma:
                        nc.vector.tensor_copy(yt[:, n0:n0 + nw], yps[:, 0:nw])
                    else:
                        yt = ysb.tile([P, NC5], f32, tag="yt")
                        nc.vector.tensor_copy(yt[:, 0:nw], yps[:, 0:nw])
                        nc.sync.dma_start(
                            y[m * P:(m + 1) * P, n0:n0 + nw], yt[:, 0:nw])
                if y_pair_dma:
                    nc.sync.dma_start(y[m * P:(m + 1) * P, :], yt[:])

        # Emission schedule: per-engine instruction order is static after
        # scheduling, so projection granules are threaded between attention
        # (h, ih) phases — each phase's inputs emitted one phase ahead; the
        # ACT-paced attention then hides the remaining projection PE work.
        attn_phases = [(h, ih) for h in range(nh) for ih in range(n_ih)]

        def phase_needs(idx):
            # granules that must be emitted before attention phase idx;
            # every phase's j-loop consumes ALL v tiles, so v has deadline 0.
            if idx >= len(attn_phases):
                return []
            h, ih = attn_phases[idx]
            mt = h // heads_per_mtile
            need = [("k", mt, n0, nw) for n0, nw in chunks(seq)]
            need += [("q", mt, n0, nw) for n0, nw in chunks(seq)
                     if n0 < (ih + 1) * ihw and n0 + nw > ih * ihw]
            if idx == 0:
                need += [("v", jt) for jt in range(seqt)]
            return need

        emitted = set()

        def emit_granules(needs):
            for g in needs:
                if g in emitted:
                    continue
                emitted.add(g)
                if g[0] == "v":
                    proj_v_tile(g[1])
                else:
                    which = 0 if g[0] == "k" else 1
                    proj_kq_tile(g[1], which, g[2], g[3])

        all_granules = []
        for idx in range(len(attn_phases)):
            for g in phase_needs(idx):
                if g not in all_granules:
                    all_granules.append(g)

        if phases == 'proj':
            emit_granules(all_granules)
        else:
            emit_granules(phase_needs(0))
            # deadline-ordered backlog, spread evenly across early boundaries
            backlog = [g for g in all_granules if g not in emitted]
            nb = max(1, len(attn_phases) - 2)
            share = -(-len(backlog) // nb)
            last = len(attn_phases) - 1
            for idx, (h, ih) in enumerate(attn_phases):
                def _cb(idx=idx):
                    emit_granules(phase_needs(idx + 1))
                    take = [g for g in backlog if g not in emitted][:share]
                    emit_granules(take)
                attn_head_ih(h, ih, nm_override="ln" if idx == last else None,
                             pre_norm_cb=_cb)
                if phases == 'all' and idx == last - 1 and n_ih > 1:
                    # final-proj m-tiles whose i-range completes at the
                    # second-to-last phase overlap the last phase's attention
                    lh, lih = attn_phases[last]
                    done_ih = [p_ih for p_ih in range(n_ih) if p_ih != lih]
                    ms = [m for m in range(seqt)
                          if (m * P) // ihw in done_ih]
                    final_proj(ms)
            if phases == 'all':
                lh, lih = attn_phases[last]
                if n_ih > 1:
                    final_proj([m for m in range(seqt)
                                if (m * P) // ihw == lih])
                else:
                    final_proj()

    nc.compile()
    return nc


_NC_CACHE = {}


def _get_nc():
    if "nc" not in _NC_CACHE:
        _NC_CACHE["nc"] = build_nc()
    return _NC_CACHE["nc"]


def _prep_core_inputs(x, Wq, Wkv, Wo):
    """Host-side shard + layout prep: per-core fp16 slices."""
    f16 = np.float16
    in_maps = []
    for c in range(N_CORES):
        b, g = c // 2, c % 2
        s = slice(g * DMC, (g + 1) * DMC)
        in_maps.append({
            "xt": np.ascontiguousarray(x[b].T).astype(f16),
            "wq": np.ascontiguousarray(Wq[:, s]).astype(f16),
            "wk": np.ascontiguousarray(Wkv[:, g * DMC:(g + 1) * DMC]).astype(f16),
            "wv": np.ascontiguousarray(
                Wkv[:, DIM_MODEL + g * DMC:DIM_MODEL + (g + 1) * DMC]).astype(f16),
            "wo": np.ascontiguousarray(Wo[s, :]).astype(f16),
        })
    return in_maps


def kernel(x, Wq, Wkv, Wo, bo):
    from concourse import bass_utils

    x = np.asarray(x, dtype=np.float32)
    Wq = np.asarray(Wq, dtype=np.float32)
    Wkv = np.asarray(Wkv, dtype=np.float32)
    Wo = np.asarray(Wo, dtype=np.float32)
    bo = np.asarray(bo, dtype=np.float32)

    nc = _get_nc()
    in_maps = _prep_core_inputs(x, Wq, Wkv, Wo)
    res = bass_utils.run_bass_kernel_spmd(nc, in_maps,
                                          core_ids=list(range(N_CORES)))
    out = np.empty((B, N, QDIM), dtype=np.float32)
    for b in range(B):
        out[b] = res.results[2 * b]["y"] + res.results[2 * b + 1]["y"] + bo
    return out
